# revision 16
# baseline (speedup 1.0000x reference)
"""Trainium2 Bass kernel for nn_Attention_18726057410905.

Multi-head causal attention: B=8, S=1024, D=768, N=12 heads, H=64.
Sharding: data-parallel over batch -- core b computes batch element b.
No collectives.

Per-core dataflow (matmul inputs bf16, fp32 PSUM accumulation):
  x^T   [d,s]   via PE identity-transposes of fp32 x (4 tiles per PSUM
        quad); ACT evacuates each quad to bf16 xT
  Q^T,K^T [2*64h, s] per head-pair, weight-stationary over both 512-col
        q-blocks (each stationary W tile serves 2 matmuls)
  V_aug [s, n, 128]  natural layout + 64-wide ones block (cols 64:128)
  S^T   [k-tile 128, 2 halves x 512q] -- one 2-bank PSUM tile per k-tile,
        2 heads row-packed on the PE (K=64 contraction)
  P^T   = exp(S^T/8) via one ACT activation per k-tile; triangular mask
        (gpsimd) on diagonal tiles only; fully-masked tiles never computed
  z_aug^T [128, q] = sum_k V_aug.T @ P^T; rows 64:128 hold the softmax
        denominators replicated by the ones block (broadcast for free)
  z^T normalized on DVE: reciprocal_approx_fast straight from PSUM, then
        one multiply writing bf16 zT
  out   [q, e] = z^T.T @ W_O + b_O, zT-stationary (2 matmuls per ldw)

Schedule: j-outer two-phase attention. Phase C runs j=0 (first 512 q)
for all six head pairs with the remaining Q/K/V projections drip-fed
into the PE stream as filler; phase D runs j=1 with the q<512 output
projection dripped in, so output DMA overlaps the second attention half.
Fill pacing is cycle-weighted with data-dependency fences so the
in-order PE never emits a consumer ahead of its producer.
"""

from collections import deque
from contextlib import ExitStack

import numpy as np

import concourse.bass as bass
import concourse.tile as tile
from concourse import bacc, mybir
from concourse.bass_utils import run_bass_kernel_spmd
from concourse.masks import make_identity, make_upper_triangular

B, S, D, N, H = 8, 1024, 768, 12, 64
P = 128
N_CORES = 8
DT = D // P          # 6 d-tiles
NPAIR = N // 2       # 6 head pairs
QB = 512             # q-block width
SB = S // QB         # 2 q/s blocks
KT = S // P          # 8 k/s tiles
EB = 384             # e-block width for V/O projection halves
LOOKAHEAD = 4        # k-tiles of PV deferral (keeps PE fed while ACT exps)
BF16 = mybir.dt.bfloat16
F32 = mybir.dt.float32
AF = mybir.ActivationFunctionType
ALU = mybir.AluOpType


def _build_nc():
    nc = bacc.Bacc(
        "TRN2", target_bir_lowering=False, debug=False, num_devices=N_CORES
    )
    x_d = nc.dram_tensor("x", [S, D], F32, kind="ExternalInput").ap()
    wq_d = nc.dram_tensor("wq", [N, D, H], F32, kind="ExternalInput").ap()
    wk_d = nc.dram_tensor("wk", [N, D, H], F32, kind="ExternalInput").ap()
    wv_d = nc.dram_tensor("wv", [N, D, H], F32, kind="ExternalInput").ap()
    wo_d = nc.dram_tensor("wo", [N, H, D], F32, kind="ExternalInput").ap()
    bq_d = nc.dram_tensor("bq", [N, H], F32, kind="ExternalInput").ap()
    bk_d = nc.dram_tensor("bk", [N, H], F32, kind="ExternalInput").ap()
    bv_d = nc.dram_tensor("bv", [N, H], F32, kind="ExternalInput").ap()
    bo_d = nc.dram_tensor("bo", [D], F32, kind="ExternalInput").ap()
    out_d = nc.dram_tensor("out", [S, D], F32, kind="ExternalOutput").ap()

    with tile.TileContext(nc) as tc, ExitStack() as ctx:
        _body(ctx, tc, x_d, wq_d, wk_d, wv_d, wo_d, bq_d, bk_d, bv_d, bo_d, out_d)
    nc.compile()
    return nc


def _body(ctx, tc, x_d, wq_d, wk_d, wv_d, wo_d, bq_d, bk_d, bv_d, bo_d, out_d):
    nc = tc.nc
    const = ctx.enter_context(tc.tile_pool(name="const", bufs=1))
    wstage = ctx.enter_context(tc.tile_pool(name="wstage", bufs=3))
    xstage = ctx.enter_context(tc.tile_pool(name="xstage", bufs=8))
    ppool = ctx.enter_context(tc.tile_pool(name="ppool", bufs=8))
    spool = ctx.enter_context(tc.tile_pool(name="spool", bufs=2))
    opool = ctx.enter_context(tc.tile_pool(name="opool", bufs=2))
    ps_mm = ctx.enter_context(tc.tile_pool(name="ps_mm", bufs=3, space="PSUM"))
    ps_z = ctx.enter_context(tc.tile_pool(name="ps_z", bufs=2, space="PSUM"))

    # --- engine warmups ----------------------------------------------------
    # DVE pays ~11us on its first real op; ACT pays a ~2.7us exp-table load.
    # Absorb both at t=0, concurrent with the input DMAs.
    warm = const.tile([1, 8], F32, tag="warm")
    nc.vector.memset(warm[:], 1.0)
    warmp = ps_z.tile([1, 8], F32, tag="z", name="warmp")
    nc.vector.tensor_copy(warmp[:], warm[:])
    warmb = const.tile([1, 8], BF16, tag="warmb")
    nc.vector.tensor_copy(warmb[:], warmp[:])  # preload DVE psum-read CAST path
    nc.scalar.activation(warm[:], warm[:], AF.Exp, scale=1.0)

    # --- constants first on gpsimd (ident gates the first PE transpose) ---
    ident = const.tile([P, P], F32, tag="ident")
    make_identity(nc, ident[:])
    # trimask[r, c] = 1 if r <= c else 0 (keep k <= q in [k, q] layout)
    trimask = const.tile([P, P], BF16, tag="trimask")
    make_upper_triangular(nc, trimask[:], val=1.0, diag=True)

    # --- input DMAs (issue order = arrival order) -------------------------
    # sync:   x st0-3, wv dt0-2, (later) out rows
    # vector: x st4-7, wv dt3-5
    # gpsimd: biases, wq/wk pair stages, v_aug ones memset, wo stages
    # scalar: no DMA duty -- ACT is needed for xT evacuation + exp
    xs_tiles = []
    for st in range(KT):
        xs = xstage.tile([P, D], F32, tag="xs", name=f"xs{st}")
        eng = nc.sync if st < 4 else nc.scalar
        eng.dma_start(xs[:], x_d[bass.ts(st, P), :])
        xs_tiles.append(xs)

    bq_sb = const.tile([P, NPAIR], F32, tag="bq")
    nc.gpsimd.dma_start(bq_sb[:], bq_d.rearrange("(pr two) h -> (two h) pr", two=2))
    bk_sb = const.tile([P, NPAIR], F32, tag="bk")
    nc.gpsimd.dma_start(bk_sb[:], bk_d.rearrange("(pr two) h -> (two h) pr", two=2))
    bv_rep = const.tile([P, N * H], F32, tag="bvrep")
    nc.gpsimd.dma_start(
        bv_rep[:], bv_d.rearrange("n h -> (n h)")[None, :].to_broadcast((P, N * H))
    )
    bo_rep = const.tile([P, D], F32, tag="borep")
    nc.gpsimd.dma_start(bo_rep[:], bo_d[None, :].to_broadcast((P, D)))

    # wq/wk layout: [N, D, H] -> [128 dp, NPAIR, DT, (n2 h)]
    wq_sb = const.tile([P, NPAIR, DT, P], BF16, tag="wq")
    wk_sb = const.tile([P, NPAIR, DT, P], BF16, tag="wk")
    wq_r = wq_d.rearrange("n (dt dp) h -> n dp dt h", dp=P)
    wk_r = wk_d.rearrange("n (dt dp) h -> n dp dt h", dp=P)
    qk_stages = []

    def stage_qk(pr):
        stgs = []
        for w_r, nm in ((wq_r, "q"), (wk_r, "k")):
            stg = wstage.tile(
                [P, DT, P], F32, tag="wpstg", name=f"stg{nm}{pr}", bufs=3
            )
            for a in range(2):
                nc.gpsimd.dma_start(stg[:, :, bass.ts(a, H)], w_r[2 * pr + a])
            stgs.append(stg)
        qk_stages.append(stgs)

    for pr in range(3):
        stage_qk(pr)

    # V_aug ones block: makes the PV matmul replicate the softmax
    # denominators into PSUM partitions 64:128 -- broadcast for free.
    v_aug = const.tile([P, KT, N, 2 * H], BF16, tag="vaug")
    nc.gpsimd.memset(v_aug[:], 1.0)

    for pr in range(3, NPAIR):
        stage_qk(pr)

    # wv: [N, D, H] -> [128 dp, DT, (pr n2 h)], staged per dt
    wv_sb = const.tile([P, DT, N * H], BF16, tag="wv")
    wv_r = wv_d.rearrange("(pr a) (dt dp) h -> dt dp pr a h", a=2, dp=P)
    wv_stages = []
    for dt in range(DT):
        stg = wstage.tile([P, N * H], F32, tag="wstg", name=f"stgv{dt}", bufs=6)
        eng = nc.sync if dt < 3 else nc.scalar
        eng.dma_start(
            stg[:].rearrange("p (pr a b) -> p pr a b", pr=NPAIR, a=2), wv_r[dt]
        )
        wv_stages.append(stg)

    # wo: [N, H, D] -> [128 (n2 h), NPAIR, D], staged per pair
    wo_sb = const.tile([P, NPAIR, D], BF16, tag="wo")
    wo_stages = []
    for pr in range(NPAIR):
        stg = wstage.tile([P, D], F32, tag="wostg", name=f"stgo{pr}", bufs=2)
        nc.gpsimd.dma_start(
            stg[:], wo_d[2 * pr : 2 * pr + 2].rearrange("n h e -> (n h) e")
        )
        wo_stages.append(stg)

    # --- weight casts (DVE) ------------------------------------------------
    def cast_qk_pair(pr):
        nc.vector.tensor_copy(wq_sb[:, pr], qk_stages[pr][0][:])
        nc.vector.tensor_copy(wk_sb[:, pr], qk_stages[pr][1][:])

    # --- x transpose -> xT [128 dp, DT, S] --------------------------------
    # PE transposes fp32 x tiles directly (identity matmul), st-outer so
    # the PE starts on the first arriving x tile. Three [128, 2, 512] PSUM
    # tiles hold all 6 d-tiles of one 512-row half; ACT evacuates each
    # 512-wide quad to bf16 in one op.
    xT = const.tile([P, DT, S], BF16, tag="xT")

    def transpose_half(half):
        pts = [
            ps_mm.tile([P, 2, QB], F32, tag="mm", name=f"xtr_{half}_{k}")
            for k in range(3)
        ]
        for q in range(4):
            st = 4 * half + q
            for dt in range(DT):
                # one accumulation group per PSUM bank: start zeroes the
                # whole 2KB granule, later transposes land on zeroed bytes
                nc.tensor.matmul(
                    pts[dt // 2][:, dt % 2, bass.ts(q, P)],
                    xs_tiles[st][:, bass.ts(dt, P)],
                    ident[:],
                    is_transpose=True,
                    start=(q == 0),
                    stop=(q == 3),
                )
        for dt in range(DT):
            nc.scalar.activation(
                xT[:, dt, bass.ts(half, QB)],
                pts[dt // 2][:, dt % 2, :],
                AF.Copy,
                scale=1.0,
            )

    qT = const.tile([P, NPAIR, S], BF16, tag="qT")
    kT = const.tile([P, NPAIR, S], BF16, tag="kT")
    zT = const.tile([P, NPAIR, S], BF16, tag="zT")

    # --- filler step generators: lists of (pe_cycles, closure) ------------
    def qk_steps(pr):
        # weight-stationary: each (half, dt) W tile serves both q-blocks;
        # Q fully evacuated before K allocates (<=1 long-lived PSUM tile).
        steps = []
        box = {}
        for half in range(2):
            wsb, bsb, dstT = (
                (wq_sb, bq_sb, qT) if half == 0 else (wk_sb, bk_sb, kT)
            )
            for dt in range(DT):
                for sb_i in range(SB):
                    def go(half=half, dt=dt, sb_i=sb_i, wsb=wsb, bsb=bsb,
                           dstT=dstT):
                        if half not in box:
                            box[half] = ps_mm.tile(
                                [P, 2 * QB], F32, tag="mm",
                                name=f"pqk{pr}h{half}",
                            )
                        t = box[half]
                        nc.tensor.matmul(
                            t[:, bass.ts(sb_i, QB)],
                            lhsT=wsb[:, pr, dt, :],
                            rhs=xT[:, dt, bass.ts(sb_i, QB)],
                            start=(dt == 0),
                            stop=(dt == DT - 1),
                        )
                        if dt == DT - 1 and sb_i == SB - 1:
                            nc.vector.tensor_scalar_add(
                                dstT[:, pr, :], t[:], bsb[:, pr : pr + 1]
                            )
                            del box[half]

                    steps.append((QB, go))
        return steps

    def v_steps(st):
        # x^T-tile-stationary: each xT tile serves both 384-wide nh halves
        steps = []
        box = {}
        for dt in range(DT):
            for blk in range(2):
                def go(dt=dt, blk=blk):
                    if "t" not in box:
                        box["t"] = ps_mm.tile(
                            [P, 2 * QB], F32, tag="mm", name=f"pv{st}"
                        )
                    t = box["t"]
                    # bank-aligned halves: blk0 at [0:384], blk1 at [512:896]
                    nc.tensor.matmul(
                        t[:, blk * QB : blk * QB + EB],
                        lhsT=xT[:, dt, bass.ts(st, P)],
                        rhs=wv_sb[:, dt, bass.ts(blk, EB)],
                        start=(dt == 0),
                        stop=(dt == DT - 1),
                    )
                    if dt == DT - 1 and blk == 1:
                        src = t.rearrange("p (b c) -> p b c", b=2)[
                            :, :, 0:EB
                        ].rearrange("p b (n h) -> p b n h", h=H)
                        nc.vector.tensor_tensor(
                            v_aug[:, st, :, 0:H].rearrange(
                                "p (b n) h -> p b n h", b=2
                            ),
                            src,
                            bv_rep[:].rearrange(
                                "p (b n h) -> p b n h", b=2, h=H
                            ),
                            ALU.add,
                        )
                        del box["t"]

                steps.append((EB, go))
        return steps

    def o_steps(qt):
        # z^T-tile-stationary: each zT tile serves both 384-wide e halves;
        # one [128, 768] evac + one full-row out DMA per q-tile.
        steps = []
        box = {}
        for pr in range(NPAIR):
            for eb in range(2):
                def go(pr=pr, eb=eb):
                    if "t" not in box:
                        box["t"] = ps_mm.tile(
                            [P, 2 * QB], F32, tag="mm", name=f"po{qt}"
                        )
                    t = box["t"]
                    # bank-aligned halves: eb0 at [0:384], eb1 at [512:896]
                    nc.tensor.matmul(
                        t[:, eb * QB : eb * QB + EB],
                        lhsT=zT[:, pr, bass.ts(qt, P)],
                        rhs=wo_sb[:, pr, bass.ts(eb, EB)],
                        start=(pr == 0),
                        stop=(pr == NPAIR - 1),
                    )
                    if pr == NPAIR - 1 and eb == 1:
                        ot = opool.tile([P, D], F32, tag="ot")
                        src = t.rearrange("p (b c) -> p b c", b=2)[:, :, 0:EB]
                        nc.vector.tensor_tensor(
                            ot[:].rearrange("p (b c) -> p b c", b=2),
                            src,
                            bo_rep[:].rearrange("p (b c) -> p b c", b=2),
                            ALU.add,
                        )
                        nc.sync.dma_start(out_d[bass.ts(qt, P), :], ot[:])
                        del box["t"]

                steps.append((EB, go))
        return steps

    # --- cycle-weighted fill pacing with dependency fences ----------------
    fill_q = deque()  # (pe_cycles, closure, batch_id)
    state = {"fill": 0, "anchor": 1, "ratio": 0.0}
    next_bid = [0]

    def add_fills(steps):
        bid = next_bid[0]
        next_bid[0] += 1
        for cyc, fn in steps:
            fill_q.append((cyc, fn, bid))
        return bid

    def fence(bid):
        # force-drain every fill up to and including batch `bid`
        while fill_q and fill_q[0][2] <= bid:
            cyc, fn, _ = fill_q.popleft()
            fn()
            state["fill"] += cyc

    def fill():
        while fill_q and state["fill"] < state["anchor"] * state["ratio"]:
            cyc, fn, _ = fill_q.popleft()
            fn()
            state["fill"] += cyc

    def set_phase(tot_anchor):
        tot_fill = sum(c for c, _, _ in fill_q)
        state["fill"] = 0
        state["anchor"] = 1
        state["ratio"] = tot_fill / tot_anchor

    def anchor(cyc):
        state["anchor"] += cyc

    # --- attention for one (pair, j) --------------------------------------
    def attn(pr, j, pre_pv=None):
        n_kt = 4 * (j + 1)
        pz = []
        pts = {}

        def emit_st(i):
            q_off = max(0, (i - 4 * j) * P)
            ps = ps_mm.tile([P, 2 * QB], F32, tag="mm", name=f"s_{pr}_{j}_{i}")
            for half in range(2):
                lo = 64 * half
                nc.tensor.matmul(
                    ps[:, half * QB + q_off : (half + 1) * QB],
                    lhsT=kT[lo : lo + 64, pr, bass.ts(i, P)],
                    rhs=qT[lo : lo + 64, pr, j * QB + q_off : (j + 1) * QB],
                    start=True,
                    stop=True,
                )
            pT = ppool.tile([P, 2, QB], BF16, tag="pT")
            ps3 = ps.rearrange("p (h q) -> p h q", h=2)
            nc.scalar.activation(
                pT[:, :, q_off:], ps3[:, :, q_off:], AF.Exp, scale=0.125
            )
            if i >= 4 * j:  # diagonal tile: triangular mask, both halves
                nc.gpsimd.tensor_tensor(
                    pT[:, :, q_off : q_off + P],
                    pT[:, :, q_off : q_off + P],
                    trimask[:, None, :].to_broadcast((P, 2, P)),
                    ALU.mult,
                )
            pts[i] = pT
            anchor(2 * (QB - q_off))

        def emit_pv(i):
            if not pz:
                if pre_pv is not None:
                    pre_pv()
                pz.extend(
                    ps_z.tile([P, QB], F32, tag="z", name=f"z_{pr}_{j}_{h}")
                    for h in range(2)
                )
            q_off = max(0, (i - 4 * j) * P)
            for half in range(2):
                n = 2 * pr + half
                nc.tensor.matmul(
                    pz[half][:, q_off:],
                    lhsT=v_aug[:, i, n, :],
                    rhs=pts[i][:, half, q_off:],
                    start=(i == 0),
                    stop=(i == n_kt - 1),
                )
            anchor(2 * (QB - q_off))

        for i in range(n_kt):
            emit_st(i)
            fill()
            if i >= LOOKAHEAD:
                emit_pv(i - LOOKAHEAD)
                fill()
        for i in range(max(0, n_kt - LOOKAHEAD), n_kt):
            emit_pv(i)
            fill()

        # normalize z^T (DVE): reciprocal straight from PSUM rows 64:128
        # (denominators, replicated there by the ones block), then multiply.
        for half in range(2):
            lo = 64 * half
            sm = spool.tile([64, QB], F32, tag="sm")
            nc.vector.tensor_copy(sm[:], pz[half][H : 2 * H, :])
            rc = spool.tile([64, QB], F32, tag="rc")
            nc.vector.reciprocal_approx_fast(rc[:], sm[:])
            nc.vector.tensor_mul(
                zT[lo : lo + 64, pr, bass.ts(j, QB)], pz[half][0:H, :], rc[:]
            )

    # --- phase A/B: transposes + pair-0 projection + weight casts ---------
    cast_qk_pair(0)
    p0_steps = qk_steps(0)
    transpose_half(0)
    for _, go in p0_steps[: 2 * DT]:  # Q half (q-block 0 + 1)
        go()
    transpose_half(1)
    for _, go in p0_steps[2 * DT :]:  # K half
        go()
    for dt in range(DT):
        nc.vector.tensor_copy(wv_sb[:, dt, :], wv_stages[dt][:])
    for pr in range(1, NPAIR):
        cast_qk_pair(pr)

    def cast_wo():
        for pr in range(NPAIR):
            nc.vector.tensor_copy(wo_sb[:, pr, :], wo_stages[pr][:])

    # --- phase C: j=0 attention, projections dripped in -------------------
    v03_bid = add_fills([s for st in range(4) for s in v_steps(st)])
    qk_bid = {1: add_fills(qk_steps(1))}
    add_fills([(0, cast_wo)])
    qk_bid[2] = add_fills(qk_steps(2))
    v47_bid = add_fills([s for st in range(4, KT) for s in v_steps(st)])
    for pr in range(3, NPAIR):
        qk_bid[pr] = add_fills(qk_steps(pr))
    set_phase(NPAIR * 2 * 2 * (QB + 3 * QB // 4 + QB // 2 + QB // 4))
    attn(0, 0, pre_pv=lambda: fence(v03_bid))
    for pr in range(1, NPAIR):
        fence(qk_bid[pr])
        attn(pr, 0)

    # --- phase D: j=1 attention, q<512 output projection dripped in -------
    fence(v47_bid)
    for qt in range(4):
        add_fills(o_steps(qt))
    set_phase(NPAIR * 2 * 2 * (5 * QB + 3 * QB // 4 + QB // 2 + QB // 4))
    for pr in range(NPAIR):
        attn(pr, 1)
    while fill_q:
        _, fn, _ = fill_q.popleft()
        fn()

    # --- output projection tail (q-tiles 4-7) -----------------------------
    for qt in range(4, KT):
        for _, go in o_steps(qt):
            go()


_CACHE = {}


def get_nc():
    if "nc" not in _CACHE:
        _CACHE["nc"] = _build_nc()
    return _CACHE["nc"]


def kernel(normalized_resid_pre, W_Q, W_K, W_V, W_O, b_Q, b_K, b_V, b_O, **kw):
    x = np.ascontiguousarray(np.asarray(normalized_resid_pre, dtype=np.float32))
    shared = {
        "wq": np.ascontiguousarray(np.asarray(W_Q, dtype=np.float32)),
        "wk": np.ascontiguousarray(np.asarray(W_K, dtype=np.float32)),
        "wv": np.ascontiguousarray(np.asarray(W_V, dtype=np.float32)),
        "wo": np.ascontiguousarray(np.asarray(W_O, dtype=np.float32)),
        "bq": np.ascontiguousarray(np.asarray(b_Q, dtype=np.float32)),
        "bk": np.ascontiguousarray(np.asarray(b_K, dtype=np.float32)),
        "bv": np.ascontiguousarray(np.asarray(b_V, dtype=np.float32)),
        "bo": np.ascontiguousarray(np.asarray(b_O, dtype=np.float32)),
    }
    in_maps = [dict(shared, x=x[b]) for b in range(B)]
    nc = get_nc()
    res = run_bass_kernel_spmd(nc, in_maps, core_ids=list(range(N_CORES)))
    return np.stack([res.results[b]["out"] for b in range(B)], axis=0)
